# revision 21
# baseline (speedup 1.0000x reference)
"""Trainium2 Bass kernel for NeuralAggregation (gnn_message_passing).

Computation (reference):
    proj = features @ W                      # [N, D] fp32
    amax = max(adjacency, axis=1)            # [N, 1]
    amin = min(adjacency, axis=1)            # [N, 1]
    out  = max(amax*proj, amin*proj, 0)

adjacency is uniform[0,1) so amin >= 0 and amax >= amin >= 0, hence
    max(amax*p, amin*p, 0) == relu(amax * p)   elementwise
(p >= 0 -> amax*p is the max and is >= 0; p < 0 -> both products <= 0).
The kernel therefore computes relu(amax * (features @ W)).

This problem is memory-bound: per core the minimum HBM traffic is the
feature read + adjacency read + output write. Features/W/output move in
fp16 (host converts; output upconverted on the host) and adjacency as
uint8 (host encodes round(a*255); device dequantizes amax by 1/255),
cutting traffic to 13.25 MB/core vs 27.2 fp32 and running the PE at
1 cycle/row instead of 4. End-to-end L2 error is ~1.2e-3 vs the 2e-2
gate.

Sharding: rows (nodes) split across 8 cores, W replicated. Host-side
prep packs each feature shard as [128, NBLK*2*BLOCK] so a block's DMA
is one contiguous 7KB run per partition (128 descriptors); the output
uses the mirrored [128, TILES*DIM] layout, un-permuted on the host.

Queue layout (measured on this part): DMA bandwidth is capped per
DIRECTION (~175 GB/s each way) and reads/writes stream concurrently
when kept on separate HWDGE queues, while SWDGE (gpsimd) descriptor
generation (~1us/transfer) poisons bulk paths. So ALL loads (adjacency
+ per-block features) ride SP/HWDGE and ALL stores ride the Act
engine's HWDGE queue, where each block's output write is naturally
ordered after the activations Act itself produced.

Per 1792-node block (7 blocks/core): DVE max-reduces that block's
adjacency columns (split per block so DVE never blocks the PSUM drain
for long), then per 128-node sub-tile two accumulating fp16 matmuls
and one fused scale+ReLU, alternating between the Act engine
(activation, per-partition amax scale) and DVE (tensor_scalar
mult+max) — neither engine alone keeps up with the ~400ns/sub-tile
pace. 8 PSUM banks keep the PE continuously fed so it stays at full
p-state clock.
"""

import numpy as np
from contextlib import ExitStack

# Problem constants (hardcoded per task contract).
N_NODES = 100000
DIM = 256
DEG = 32
N_CORES = 8
SH = 12544            # padded rows per core  (98 tiles of 128)
N_PAD = SH * N_CORES  # 100352
TILES = SH // 128     # 98
BT = 14               # 128-row sub-tiles per block
NBLK = TILES // BT    # 7
BLOCK = BT * 128      # 1792

# IO/matmul dtype: "f16" (default) or "f32" (exact fallback for A/B).
MM_DTYPE = "f16"
W_NP_DTYPE = np.float16
# adjacency as uint8 (host encodes round(a*255), device dequants amax/255):
# halves adjacency traffic; amax quantization error <= 0.2% absolute.
ADJ_U8 = True
# split the adjacency load across both HWDGE queues (4 blocks' worth on SP,
# 3 on Act) to balance duplex direction traffic at ~6.6MB each way.
ADJ_SPLIT = False

_NC_CACHE = {}


def _build_nc(repeat=1, trace_sim=False, mm_dtype=None, timing=False):
    """Build the per-core Bass program (identical on all 8 cores).

    timing=True builds a variant whose big tensors live in Internal DRAM
    (no host transfer) with the pipeline wrapped in a For_i(repeat) loop;
    used only for measurement, not for results.
    """
    import concourse.tile as tile
    from concourse import bacc, mybir

    f32 = mybir.dt.float32
    mm_dtype = mm_dtype or MM_DTYPE
    dt_io = {"f32": f32, "f16": mybir.dt.float16}[mm_dtype]
    dt_adj = mybir.dt.uint8 if ADJ_U8 else dt_io
    Relu = mybir.ActivationFunctionType.Relu

    FW = NBLK * 2 * BLOCK   # feature row length per partition
    OW = TILES * DIM        # output row length per partition

    nc = bacc.Bacc("TRN2", target_bir_lowering=False, debug=False)
    if timing:
        featP = nc.dram_tensor("featP_i", [128, FW], dt_io).ap()
        adjR = nc.dram_tensor("adjR_i", [128, TILES * DEG], dt_adj).ap()
        out = nc.dram_tensor("out_i", [128, OW], dt_io).ap()
        wR = nc.dram_tensor("wR", [128, 2 * DIM], dt_io, kind="ExternalInput").ap()
        tiny = nc.dram_tensor("tiny", [128, 4], f32, kind="ExternalOutput").ap()
    else:
        featP = nc.dram_tensor("featP", [128, FW], dt_io, kind="ExternalInput").ap()
        adjR = nc.dram_tensor("adjR", [128, TILES * DEG], dt_adj, kind="ExternalInput").ap()
        wR = nc.dram_tensor("wR", [128, 2 * DIM], dt_io, kind="ExternalInput").ap()
        out = nc.dram_tensor("out", [128, OW], dt_io, kind="ExternalOutput").ap()

    with tile.TileContext(nc, trace_sim=trace_sim) as tc, ExitStack() as ctx:
        const_pool = ctx.enter_context(tc.tile_pool(name="const", bufs=1))
        ft_pool = ctx.enter_context(tc.tile_pool(name="ft", bufs=4))
        adj_pool = ctx.enter_context(tc.tile_pool(name="adj", bufs=2))
        amax_pool = ctx.enter_context(tc.tile_pool(name="amax", bufs=2))
        out_pool = ctx.enter_context(tc.tile_pool(name="outp", bufs=7))
        ps_pool = ctx.enter_context(tc.tile_pool(name="ps", bufs=8, space="PSUM"))

        w_sb = const_pool.tile([128, 2 * DIM], dt_io)
        nc.sync.dma_start(w_sb[:], wR[:])

        def body():
            # One adjacency DMA per pass; the max-reduce is split per block
            # so DVE is never occupied more than ~0.5us at a stretch (a
            # single whole-shard reduce stalls the odd-tile PSUM drain and
            # with it the PE). All loads ride the SP/HWDGE queue: measured
            # aggregate DMA bandwidth is the constraint, and SWDGE (gpsimd)
            # descriptor generation costs ~1us per transfer.
            adj = adj_pool.tile([128, TILES * DEG], dt_adj, tag="adj")
            if ADJ_SPLIT:
                cut = 4 * BT * DEG  # first 4 blocks' columns
                nc.sync.dma_start(adj[:, :cut], adjR[:, :cut])
                nc.scalar.dma_start(adj[:, cut:], adjR[:, cut:])
            else:
                nc.sync.dma_start(adj[:], adjR[:])
            amax = amax_pool.tile([128, TILES], f32, tag="amax")

            for b in range(NBLK):
                ft = ft_pool.tile([128, 2 * BLOCK], dt_io, tag="ft")
                nc.sync.dma_start(ft[:], featP[:, b * 2 * BLOCK : (b + 1) * 2 * BLOCK])

                asl = amax[:, b * BT : (b + 1) * BT]
                nc.vector.tensor_reduce(
                    asl,
                    adj[:, b * BT * DEG : (b + 1) * BT * DEG].rearrange(
                        "p (t j) -> p t j", j=DEG
                    ),
                    axis=mybir.AxisListType.X,
                    op=mybir.AluOpType.max,
                )
                if ADJ_U8:
                    # dequantize: amax holds round(a*255) as f32; scale back
                    nc.vector.tensor_scalar(
                        asl, asl, 1.0 / 255.0, None, op0=mybir.AluOpType.mult
                    )

                out_t = out_pool.tile([128, BT * DIM], dt_io, tag="out_t")
                for nt in range(BT):
                    ps = ps_pool.tile([128, DIM], f32, tag="ps")
                    lhs0 = ft[:, nt * 128 : nt * 128 + 128]
                    lhs1 = ft[:, BLOCK + nt * 128 : BLOCK + nt * 128 + 128]
                    nc.tensor.matmul(ps[:], lhs0, w_sb[:, 0:DIM], start=True, stop=False)
                    nc.tensor.matmul(ps[:], lhs1, w_sb[:, DIM : 2 * DIM], start=False, stop=True)
                    osl = out_t[:, nt * DIM : (nt + 1) * DIM]
                    sc = amax[:, b * BT + nt : b * BT + nt + 1]
                    if nt % 2 == 0:
                        # scale+relu on the Act engine for even sub-tiles...
                        nc.scalar.activation(osl, ps[:], Relu, bias=0.0, scale=sc)
                    else:
                        # ...and on DVE for odd ones: neither engine alone can
                        # keep up with the DMA roofline at ~400ns/sub-tile.
                        nc.vector.tensor_scalar(
                            osl, ps[:], sc, 0.0,
                            op0=mybir.AluOpType.mult, op1=mybir.AluOpType.max,
                        )

                # output leaves via the Act engine's HWDGE queue: it is
                # ordered right after the activations Act itself produced,
                # and keeps the compute-dependent write off the SP prefetch
                # stream and off the slow SWDGE path.
                nc.scalar.dma_start(out[:, b * BT * DIM : (b + 1) * BT * DIM], out_t[:])

        if timing:
            with tc.For_i(0, repeat, 1, staggered_reset=True):
                body()
            nc.sync.dma_start(tiny[:], w_sb[:, 0 : 16 // mybir.dt.size(dt_io)].bitcast(f32))
        else:
            for _ in range(repeat):
                body()
    nc.compile()
    return nc


def _get_nc(repeat=1, mm_dtype=None, timing=False):
    key = (repeat, mm_dtype or MM_DTYPE, timing)
    nc = _NC_CACHE.get(key)
    if nc is None:
        nc = _build_nc(repeat, mm_dtype=mm_dtype, timing=timing)
        _NC_CACHE[key] = nc
    return nc


def prep_inputs(features, adjacency, W, mm_dtype=None):
    """Host-side shard + relayout. Returns in_maps for the 8 cores."""
    mm_dtype = mm_dtype or MM_DTYPE
    np_io = {"f32": np.float32, "f16": np.float16}[mm_dtype]
    features = np.asarray(features, dtype=np.float32)
    adjacency = np.asarray(adjacency, dtype=np.float32)
    W = np.asarray(W, dtype=np.float32)

    fpad = np.zeros((N_PAD, DIM), dtype=np_io)
    fpad[:N_NODES] = features
    if ADJ_U8:
        apad = np.zeros((N_PAD, DEG), dtype=np.uint8)
        apad[:N_NODES] = np.rint(adjacency * 255.0).astype(np.uint8)
    else:
        apad = np.zeros((N_PAD, DEG), dtype=np_io)
        apad[:N_NODES] = adjacency

    wR = np.ascontiguousarray(
        W.reshape(2, 128, DIM).transpose(1, 0, 2).reshape(128, 2 * DIM)
    ).astype(np_io)

    in_maps = []
    for c in range(N_CORES):
        fs = fpad[c * SH : (c + 1) * SH]                      # [SH, DIM]
        # featP[p, b, cc, n] = features[b*BLOCK + n, cc*128 + p]
        featP = np.ascontiguousarray(
            fs.T.reshape(2, 128, NBLK, BLOCK)
            .transpose(1, 2, 0, 3)
            .reshape(128, NBLK * 2 * BLOCK)
        )
        ash = apad[c * SH : (c + 1) * SH]                     # [SH, DEG]
        adjR = np.ascontiguousarray(
            ash.reshape(TILES, 128, DEG).transpose(1, 0, 2).reshape(128, TILES * DEG)
        )
        in_maps.append({"featP": featP, "adjR": adjR, "wR": wR})
    return in_maps


def run_shards(in_maps, repeat=1, mm_dtype=None):
    """Run the bass kernel on the 8 cores; returns list of [SH, DIM] fp32."""
    from concourse.bass_utils import run_bass_kernel_spmd

    nc = _get_nc(repeat, mm_dtype=mm_dtype)
    res = run_bass_kernel_spmd(nc, in_maps, list(range(N_CORES)))
    outs = []
    for c in range(N_CORES):
        o = np.asarray(res.results[c]["out"])                 # [128, TILES*DIM]
        o = o.reshape(128, TILES, DIM).transpose(1, 0, 2).reshape(SH, DIM)
        outs.append(o.astype(np.float32))
    return outs


def kernel(features, adjacency, W):
    features = np.asarray(features, dtype=np.float32)
    adjacency = np.asarray(adjacency, dtype=np.float32)
    W = np.asarray(W, dtype=np.float32)
    assert features.shape == (N_NODES, DIM), features.shape
    assert adjacency.shape == (N_NODES, DEG), adjacency.shape
    assert W.shape == (DIM, DIM), W.shape

    if adjacency.min() < 0.0 or (ADJ_U8 and adjacency.max() >= 1.0):
        # The device kernel uses max(amax*p, amin*p, 0) == relu(amax*p),
        # valid because adjacency is uniform[0,1) (amin >= 0; the uint8
        # encode additionally needs a < 1). Inputs outside that
        # distribution fall back to an exact host path.
        proj = features @ W
        amax = adjacency.max(axis=1, keepdims=True)
        amin = adjacency.min(axis=1, keepdims=True)
        return np.maximum(np.maximum(amax * proj, amin * proj), 0.0).astype(np.float32)

    in_maps = prep_inputs(features, adjacency, W)
    outs = run_shards(in_maps)
    full = np.concatenate(outs, axis=0)[:N_NODES]
    return np.ascontiguousarray(full, dtype=np.float32)
